# revision 5
# baseline (speedup 1.0000x reference)
"""Trainium2 Bass kernel for nn_BoneLinear: out = x @ W^T + pooled(x) @ disha.

Identity used: pooled(x) @ disha == x @ A where A[j, o] = disha[j % 64, o]
(vertical tiling of disha).  So the whole module is one dense matmul:
    out = x @ (W^T + tile(disha, 16))

Per-core pipeline (all 8 cores run this over their own batch shard):
  1. Setup: load W naturally, PE-transpose it (4 transposes packed per PSUM
     bank as one accumulation group), add the partition-tiled disha, and round
     to fp16 -> W_eff^T resident in SBUF [128, 8, 1024].
  2. Steady state, software-pipelined over 32 token tiles: HWDGE-load x one
     1 MB token tile at a time -> cast f32->fp16 -> PE-transpose each
     [128,128] chunk (packed 4/bank) -> DVE copy to SBUF -> 16 accumulating
     fp16 matmuls (N=512) -> ACT/DVE copies PSUM->SBUF -> HWDGE store.
     Loads run EAG=4 tiles ahead and transposes TA=2 ahead so the PE stream
     (the bottleneck: 512*213ns matmuls + 256*53ns transposes per pass) never
     waits on DMA; the final tile runs its matmuls n-outer and stores in two
     half-tiles so the post-loop drain is ~2 us instead of ~5.
  fp16 operands measured relmax ~3.3e-4 vs the fp32 reference.

Sharding: pure data-parallel over batch (B=8 -> one batch element per core).
Each core reads its x shard [4096, 1024], full weight and disha, and writes
its output shard [4096, 1024].  No collectives.
"""

import sys
import os

for _p in ("/opt/trn_rl_repo", "/root/.axon_site/_ro/trn_rl_repo"):
    if os.path.isdir(_p) and _p not in sys.path:
        sys.path.insert(0, _p)

import numpy as np

import concourse.bass as bass
import concourse.mybir as mybir
import concourse.tile as tile
from concourse import bacc
from concourse.bass_utils import run_bass_kernel_spmd
from concourse.masks import make_identity

# Problem shapes (hardcoded per contract)
B, S, D_IN, D_OUT, R = 8, 4096, 1024, 1024, 64
N_CORES = 8
P = 128
KO = D_IN // P          # 8 contraction chunks of 128
OC = D_OUT // P         # 8 output chunks of 128 (for W transpose)
MT = S // P             # 32 token tiles per core
NF = 512                # matmul moving free dim (one PSUM bank of fp32)
NT = D_OUT // NF        # 2 n-tiles

F32 = mybir.dt.float32
F16 = mybir.dt.float16
MM_DT = F16


def build_bass(reps: int = 1, loop: int = 1, n_outer: bool = False,
               eag: int = 4, ta: int = 2, half_mm: bool = False):
    """reps>1 (python-unrolled) or loop>1 (hardware For_i) repeat the
    steady-state compute inside the NEFF; used only for wall-clock
    differencing in benchmarks (the graded kernel uses reps=1, loop=1).
    n_outer / half_mm are timing-diagnostic knobs only."""
    nc = bacc.Bacc("TRN2", target_bir_lowering=False, debug=False, num_devices=1)
    x_ap = nc.dram_tensor("x", [S, D_IN], F32, kind="ExternalInput").ap()
    w_ap = nc.dram_tensor("w", [D_OUT, D_IN], F32, kind="ExternalInput").ap()
    d_ap = nc.dram_tensor("disha", [R, D_OUT], F32, kind="ExternalInput").ap()
    out_ap = nc.dram_tensor("out", [S, D_OUT], F32, kind="ExternalOutput").ap()

    GRP = NF // P  # 4 transposes packed per PSUM bank

    with tile.TileContext(nc) as tc:
        with (
            tc.tile_pool(name="const", bufs=1) as const,
            tc.tile_pool(name="wp", bufs=1) as wpool,
            tc.tile_pool(name="xp", bufs=max(6, eag + 2)) as xpool,
            tc.tile_pool(name="xh", bufs=max(5, eag + 1)) as xhpool,
            tc.tile_pool(name="xtp", bufs=max(5, ta + 3)) as xtpool,
            tc.tile_pool(name="op", bufs=4) as opool,
            tc.tile_pool(name="pstp", bufs=4, space="PSUM") as psum_tp,
            tc.tile_pool(name="psacc", bufs=2, space="PSUM") as psum_acc,
        ):
            ident = const.tile([P, P], MM_DT)
            make_identity(nc, ident)

            # disha tiled twice on partitions: disha2[p, :] = disha[p % 64, :]
            disha2f = const.tile([P, D_OUT], F32)
            nc.sync.dma_start(disha2f[0:R, :], d_ap[:, :])
            nc.sync.dma_start(disha2f[R : 2 * R, :], d_ap[:, :])
            cast_copy = nc.any.tensor_copy
            disha2 = const.tile([P, D_OUT], MM_DT)
            cast_copy(disha2[:], disha2f[:])

            # Build W_eff^T[p + 128*kc, oc*128 + q] = W[q(within oc), p(of kc)] + disha2[p]
            w_eff = wpool.tile([P, KO, D_OUT], MM_DT)
            with tc.tile_pool(name="wnat", bufs=1) as wnat_pool:
                w_nat = wnat_pool.tile([P, OC, D_IN], F32)
                w_nath = wnat_pool.tile([P, OC, D_IN], MM_DT)
                w_src = w_ap.rearrange("(oc p) d -> p oc d", p=P)
                for kc in range(KO):
                    nc.sync.dma_start(
                        w_nat[:, :, kc * P : (kc + 1) * P],
                        w_src[:, :, kc * P : (kc + 1) * P],
                    )
                    cast_copy(
                        w_nath[:, :, kc * P : (kc + 1) * P],
                        w_nat[:, :, kc * P : (kc + 1) * P],
                    )
                for kc in range(KO):
                    for og in range(OC // GRP):
                        pst = psum_tp.tile([P, NF], MM_DT, tag="tp")
                        for j in range(GRP):
                            oc = og * GRP + j
                            nc.tensor.matmul(
                                pst[:, j * P : (j + 1) * P],
                                w_nath[:, oc, kc * P : (kc + 1) * P],
                                ident[:],
                                is_transpose=True,
                                start=(j == 0),
                                stop=(j == GRP - 1),
                            )
                        nc.vector.tensor_add(
                            w_eff[:, kc, og * NF : (og + 1) * NF],
                            pst[:],
                            disha2[:, og * NF : (og + 1) * NF],
                        )

            # Main loop over token tiles
            import contextlib

            loop_cm = (
                tc.For_i(0, loop, 1) if loop > 1 else contextlib.nullcontext()
            )
            with loop_cm:
                for rep in range(reps):

                    def emit_load(m, rep=rep):
                        """DMA one token tile and cast to fp16."""
                        x_t = xpool.tile(
                            [P, D_IN], F32, tag="x_t", name=f"x_{rep}_{m}"
                        )
                        nc.sync.dma_start(
                            x_t[:], x_ap[m * P : (m + 1) * P, :]
                        )
                        x_h = xhpool.tile(
                            [P, D_IN], MM_DT, tag="x_h", name=f"xh_{rep}_{m}"
                        )
                        cast_copy(x_h[:], x_t[:])
                        return x_h

                    def emit_transpose(x_h, m, rep=rep):
                        """PE-transpose token tile m."""
                        xT = xtpool.tile(
                            [P, KO, P], MM_DT, tag="xT", name=f"xT_{rep}_{m}"
                        )
                        for g in range(KO // GRP):
                            pst = psum_tp.tile(
                                [P, NF], MM_DT, tag="tp", name=f"tp_{rep}_{m}_{g}"
                            )
                            for j in range(GRP):
                                kc = g * GRP + j
                                nc.tensor.matmul(
                                    pst[:, j * P : (j + 1) * P],
                                    x_h[:, kc * P : (kc + 1) * P],
                                    ident[:],
                                    is_transpose=True,
                                    start=(j == 0),
                                    stop=(j == GRP - 1),
                                )
                            nc.vector.tensor_copy(
                                xT[:, g * GRP : (g + 1) * GRP], pst[:]
                            )
                        return xT

                    xh_buf = {}
                    xT_buf = {}
                    for j in range(min(eag, MT)):
                        xh_buf[j] = emit_load(j)
                    for j in range(min(ta, MT)):
                        xT_buf[j] = emit_transpose(xh_buf[j], j)

                    for m in range(MT):
                        if m + eag < MT:
                            xh_buf[m + eag] = emit_load(m + eag)
                        if m + ta < MT:
                            xT_buf[m + ta] = emit_transpose(
                                xh_buf[m + ta], m + ta
                            )
                        xT_cur = xT_buf.pop(m)
                        xh_buf.pop(m, None)

                        last = m == MT - 1
                        o_sb = opool.tile(
                            [P, D_OUT], F32, tag="o", name=f"o_{rep}_{m}"
                        )
                        pss = [
                            psum_acc.tile(
                                [P, NF], F32, tag=f"acc{n}", name=f"acc_{rep}_{m}_{n}"
                            )
                            for n in range(NT)
                        ]
                        if last or n_outer:
                            # n-outer: finish bank n=0's kc chain first so its
                            # copy+store overlaps the n=1 matmuls -> short drain
                            for n in range(NT):
                                for kc in range(KO):
                                    if half_mm and kc >= KO // 2:
                                        continue
                                    nc.tensor.matmul(
                                        pss[n][:],
                                        xT_cur[:, kc],
                                        w_eff[:, kc, n * NF : (n + 1) * NF],
                                        start=(kc == 0),
                                        stop=(kc == (KO // 2 if half_mm else KO) - 1),
                                    )
                                nc.any.tensor_copy(
                                    o_sb[:, n * NF : (n + 1) * NF], pss[n][:]
                                )
                                nc.sync.dma_start(
                                    out_ap[m * P : (m + 1) * P, n * NF : (n + 1) * NF],
                                    o_sb[:, n * NF : (n + 1) * NF],
                                )
                        else:
                            for kc in range(KO):
                                if half_mm and kc >= KO // 2:
                                    continue
                                for n in range(NT):
                                    nc.tensor.matmul(
                                        pss[n][:],
                                        xT_cur[:, kc],
                                        w_eff[:, kc, n * NF : (n + 1) * NF],
                                        start=(kc == 0),
                                        stop=(kc == (KO // 2 if half_mm else KO) - 1),
                                    )
                            for n in range(NT):
                                nc.any.tensor_copy(
                                    o_sb[:, n * NF : (n + 1) * NF], pss[n][:]
                                )
                            nc.sync.dma_start(
                                out_ap[m * P : (m + 1) * P, :], o_sb[:]
                            )

    nc.compile()
    return nc


def kernel(x: np.ndarray, weight: np.ndarray, disha: np.ndarray) -> np.ndarray:
    assert x.shape == (B, S, D_IN) and weight.shape == (D_OUT, D_IN)
    assert disha.shape == (R, D_OUT)
    x = np.ascontiguousarray(x, dtype=np.float32)
    weight = np.ascontiguousarray(weight, dtype=np.float32)
    disha = np.ascontiguousarray(disha, dtype=np.float32)
    in_maps = [
        {"x": x[c], "w": weight, "disha": disha} for c in range(N_CORES)
    ]
    # The axon-proxied exec occasionally dies with NRT_EXEC_UNIT_UNRECOVERABLE
    # on an otherwise-good NEFF; retry a couple of times with a fresh build.
    last_exc = None
    for attempt in range(3):
        try:
            nc = build_bass()
            res = run_bass_kernel_spmd(
                nc, in_maps, core_ids=list(range(N_CORES))
            )
            break
        except Exception as e:  # noqa: BLE001
            last_exc = e
            import time as _time

            _time.sleep(5.0 * (attempt + 1))
    else:
        raise last_exc
    out = np.stack([res.results[c]["out"] for c in range(N_CORES)], axis=0)
    return out


if __name__ == "__main__":
    rng = np.random.default_rng(0)
    x = rng.standard_normal((B, S, D_IN), dtype=np.float32)
    w = (rng.standard_normal((D_OUT, D_IN), dtype=np.float32) / 32.0).astype(
        np.float32
    )
    d = (rng.standard_normal((R, D_OUT), dtype=np.float32) * 0.01).astype(np.float32)
    out = kernel(x=x, weight=w, disha=d)
    print(out.shape, out.dtype)


# revision 11
# speedup vs baseline: 1.0710x; 1.0710x over previous
"""Trainium2 Bass kernel for nn_BoneLinear: out = x @ W^T + pooled(x) @ disha.

Identity used: pooled(x) @ disha == x @ A where A[j, o] = disha[j % 64, o]
(vertical tiling of disha).  So the whole module is one dense matmul:
    out = x @ (W^T + tile(disha, 16))

Per-core pipeline (all 8 cores run this over their own batch shard):
  1. Setup: load W naturally, PE-transpose it (4 transposes packed per PSUM
     bank as one accumulation group), add the partition-tiled disha, and round
     to fp16 -> W_eff^T resident in SBUF [128, 8, 1024].
  2. Steady state, software-pipelined over 32 token tiles: HWDGE-load x one
     1 MB token tile at a time -> cast f32->fp16 -> PE-transpose each
     [128,128] chunk (packed 4/bank) -> DVE copy to SBUF -> 16 accumulating
     fp16 matmuls (N=512) -> ACT/DVE copies PSUM->SBUF -> HWDGE store.
     Loads run EAG=4 tiles ahead and transposes TA=2 ahead so the PE stream
     (the bottleneck: 512*213ns matmuls + 256*53ns transposes per pass) never
     waits on DMA; the final tile runs its matmuls n-outer and stores in two
     half-tiles so the post-loop drain is ~2 us instead of ~5.
  fp16 operands measured relmax ~3.3e-4 vs the fp32 reference.

Sharding: pure data-parallel over batch (B=8 -> one batch element per core).
Each core reads its x shard [4096, 1024], full weight and disha, and writes
its output shard [4096, 1024].  No collectives.
"""

import sys
import os

for _p in ("/opt/trn_rl_repo", "/root/.axon_site/_ro/trn_rl_repo"):
    if os.path.isdir(_p) and _p not in sys.path:
        sys.path.insert(0, _p)

import numpy as np

import concourse.bass as bass
import concourse.mybir as mybir
import concourse.tile as tile
from concourse import bacc
from concourse.bass_utils import run_bass_kernel_spmd
from concourse.masks import make_identity


def _dedup_ldweights(ordered):
    """Drop InstLdweights that reload the stationary operand already in the
    PE array (adjacent matmuls sharing one lhsT: bass emits a fresh
    Ldweights per matmul, but the array still holds the weights, and the
    pull-ahead reorder window never runs ahead of an in-flight full-array
    matmul, so the second load is pure serial PE time ~53ns).  Only drops a
    load when the PE stream between the two loads is matmuls only; dep edges
    of the dropped load are merged into the following matmul."""
    for bb, insts in ordered.items():
        cur_sig = None
        drop = {}
        pe_ixs = [
            ix for ix, i in enumerate(insts) if i.engine == mybir.EngineType.PE
        ]
        for k, ix in enumerate(pe_ixs):
            i = insts[ix]
            t = type(i).__name__
            if t == "InstLdweights":
                sig = (
                    repr(i.ins[0]),
                    str(i.is_transpose), str(i.perf_mode),
                    str(i.tile_position), str(i.tile_size),
                )
                if sig == cur_sig and k + 1 < len(pe_ixs):
                    nxt = insts[pe_ixs[k + 1]]
                    if type(nxt).__name__ == "InstMatmult":
                        drop[ix] = nxt
                else:
                    cur_sig = sig
            elif t == "InstMatmult":
                pass  # does not disturb loaded weights
            else:
                cur_sig = None
        if drop:
            for ix, nxt in drop.items():
                nxt.add_sync_dependencies_from(
                    insts[ix].sync_dependency_set_copy()
                )
                nxt.add_nosync_dependencies_from(
                    insts[ix].nosync_dependency_set_copy()
                )
            ordered[bb] = [i for ix, i in enumerate(insts) if ix not in drop]
    return ordered

# Problem shapes (hardcoded per contract)
B, S, D_IN, D_OUT, R = 8, 4096, 1024, 1024, 64
N_CORES = 8
P = 128
KO = D_IN // P          # 8 contraction chunks of 128
OC = D_OUT // P         # 8 output chunks of 128 (for W transpose)
MT = S // P             # 32 token tiles per core
NF = 512                # matmul moving free dim (one PSUM bank of fp32)
NT = D_OUT // NF        # 2 n-tiles

F32 = mybir.dt.float32
F16 = mybir.dt.float16
MM_DT = F16


def build_bass(reps: int = 1, loop: int = 1, n_outer: bool = False,
               eag: int = 4, ta: int = 2, half_mm: bool = False,
               dedup_ldw: bool = True):
    """reps>1 (python-unrolled) or loop>1 (hardware For_i) repeat the
    steady-state compute inside the NEFF; used only for wall-clock
    differencing in benchmarks (the graded kernel uses reps=1, loop=1).
    n_outer / half_mm are timing-diagnostic knobs only."""
    _orig_legalize = tile.tile_legalize
    if dedup_ldw:
        def _legalize_and_dedup(ordered, nc_):
            return _dedup_ldweights(_orig_legalize(ordered, nc_))

        tile.tile_legalize = _legalize_and_dedup
    try:
        nc = _build_bass_inner(reps, loop, n_outer, eag, ta, half_mm)
    finally:
        tile.tile_legalize = _orig_legalize
    return nc


def _build_bass_inner(reps, loop, n_outer, eag, ta, half_mm):
    nc = bacc.Bacc("TRN2", target_bir_lowering=False, debug=False, num_devices=1)
    x_ap = nc.dram_tensor("x", [S, D_IN], F32, kind="ExternalInput").ap()
    w_ap = nc.dram_tensor("w", [D_OUT, D_IN], F32, kind="ExternalInput").ap()
    d_ap = nc.dram_tensor("disha", [R, D_OUT], F32, kind="ExternalInput").ap()
    out_ap = nc.dram_tensor("out", [S, D_OUT], F32, kind="ExternalOutput").ap()

    GRP = NF // P  # 4 transposes packed per PSUM bank

    with tile.TileContext(nc) as tc:
        with (
            tc.tile_pool(name="const", bufs=1) as const,
            tc.tile_pool(name="wp", bufs=1) as wpool,
            tc.tile_pool(name="xp", bufs=max(6, eag + 2)) as xpool,
            tc.tile_pool(name="xh", bufs=max(5, eag + 1)) as xhpool,
            tc.tile_pool(name="xtp", bufs=max(5, ta + 3)) as xtpool,
            tc.tile_pool(name="op", bufs=4) as opool,
            tc.tile_pool(name="pstp", bufs=4, space="PSUM") as psum_tp,
            tc.tile_pool(name="psacc", bufs=2, space="PSUM") as psum_acc,
        ):
            ident = const.tile([P, P], MM_DT)
            make_identity(nc, ident)

            # disha tiled twice on partitions: disha2[p, :] = disha[p % 64, :]
            disha2f = const.tile([P, D_OUT], F32)
            nc.sync.dma_start(disha2f[0:R, :], d_ap[:, :])
            nc.sync.dma_start(disha2f[R : 2 * R, :], d_ap[:, :])
            cast_copy = nc.any.tensor_copy
            disha2 = const.tile([P, D_OUT], MM_DT)
            cast_copy(disha2[:], disha2f[:])

            # Build W_eff^T[p + 128*kc, oc*128 + q] = W[q(within oc), p(of kc)] + disha2[p]
            w_eff = wpool.tile([P, KO, D_OUT], MM_DT)
            with tc.tile_pool(name="wnat", bufs=1) as wnat_pool:
                w_nat = wnat_pool.tile([P, OC, D_IN], F32)
                w_nath = wnat_pool.tile([P, OC, D_IN], MM_DT)
                w_src = w_ap.rearrange("(oc p) d -> p oc d", p=P)
                for kc in range(KO):
                    nc.sync.dma_start(
                        w_nat[:, :, kc * P : (kc + 1) * P],
                        w_src[:, :, kc * P : (kc + 1) * P],
                    )
                    cast_copy(
                        w_nath[:, :, kc * P : (kc + 1) * P],
                        w_nat[:, :, kc * P : (kc + 1) * P],
                    )
                for kc in range(KO):
                    for og in range(OC // GRP):
                        pst = psum_tp.tile([P, NF], MM_DT, tag="tp")
                        for j in range(GRP):
                            oc = og * GRP + j
                            nc.tensor.matmul(
                                pst[:, j * P : (j + 1) * P],
                                w_nath[:, oc, kc * P : (kc + 1) * P],
                                ident[:],
                                is_transpose=True,
                                start=(j == 0),
                                stop=(j == GRP - 1),
                            )
                        nc.vector.tensor_add(
                            w_eff[:, kc, og * NF : (og + 1) * NF],
                            pst[:],
                            disha2[:, og * NF : (og + 1) * NF],
                        )

            # Main loop over token tiles
            import contextlib

            loop_cm = (
                tc.For_i(0, loop, 1) if loop > 1 else contextlib.nullcontext()
            )
            with loop_cm:
                for rep in range(reps):

                    PW = 2
                    NP = MT // PW

                    def emit_load_pair(p, rep=rep):
                        """DMA a 1MB pair of token tiles and cast to fp16."""
                        x_t = xpool.tile(
                            [P, PW, D_IN], F32, tag="x_t", name=f"x_{rep}_{p}"
                        )
                        nc.sync.dma_start(
                            x_t[:],
                            x_ap[p * PW * P : (p + 1) * PW * P, :].rearrange(
                                "(two p) d -> p two d", two=PW
                            ),
                        )
                        x_h = xhpool.tile(
                            [P, PW, D_IN], MM_DT, tag="x_h", name=f"xh_{rep}_{p}"
                        )
                        cast_copy(x_h[:], x_t[:])
                        return x_h

                    def emit_transpose(x_h, m, rep=rep):
                        """PE-transpose token tile m (half m%PW of its pair)."""
                        t = m % PW
                        xT = xtpool.tile(
                            [P, KO, P], MM_DT, tag="xT", name=f"xT_{rep}_{m}"
                        )
                        for g in range(KO // GRP):
                            pst = psum_tp.tile(
                                [P, NF], MM_DT, tag="tp", name=f"tp_{rep}_{m}_{g}"
                            )
                            for j in range(GRP):
                                kc = g * GRP + j
                                nc.tensor.matmul(
                                    pst[:, j * P : (j + 1) * P],
                                    x_h[:, t, kc * P : (kc + 1) * P],
                                    ident[:],
                                    is_transpose=True,
                                    start=(j == 0),
                                    stop=(j == GRP - 1),
                                )
                            nc.vector.tensor_copy(
                                xT[:, g * GRP : (g + 1) * GRP], pst[:]
                            )
                        return xT

                    eagp = max(1, (eag + PW - 1) // PW)
                    xh_buf = {}
                    xT_buf = {}
                    for j in range(min(eagp, NP)):
                        xh_buf[j] = emit_load_pair(j)
                    for j in range(min(ta, MT)):
                        xT_buf[j] = emit_transpose(xh_buf[j // PW], j)

                    for m in range(MT):
                        if m % PW == 0 and m // PW + eagp < NP:
                            xh_buf[m // PW + eagp] = emit_load_pair(
                                m // PW + eagp
                            )
                        if m + ta < MT:
                            xT_buf[m + ta] = emit_transpose(
                                xh_buf[(m + ta) // PW], m + ta
                            )
                        xT_cur = xT_buf.pop(m)
                        if m % PW == PW - 1:
                            xh_buf.pop(m // PW, None)

                        last = m == MT - 1
                        o_sb = opool.tile(
                            [P, D_OUT], F32, tag="o", name=f"o_{rep}_{m}"
                        )
                        pss = [
                            psum_acc.tile(
                                [P, NF], F32, tag=f"acc{n}", name=f"acc_{rep}_{m}_{n}"
                            )
                            for n in range(NT)
                        ]
                        if last or n_outer:
                            # n-outer: finish bank n=0's kc chain first so its
                            # copy+store overlaps the n=1 matmuls -> short drain
                            for n in range(NT):
                                for kc in range(KO):
                                    if half_mm and kc >= KO // 2:
                                        continue
                                    nc.tensor.matmul(
                                        pss[n][:],
                                        xT_cur[:, kc],
                                        w_eff[:, kc, n * NF : (n + 1) * NF],
                                        start=(kc == 0),
                                        stop=(kc == (KO // 2 if half_mm else KO) - 1),
                                    )
                                nc.any.tensor_copy(
                                    o_sb[:, n * NF : (n + 1) * NF], pss[n][:]
                                )
                                nc.sync.dma_start(
                                    out_ap[m * P : (m + 1) * P, n * NF : (n + 1) * NF],
                                    o_sb[:, n * NF : (n + 1) * NF],
                                )
                        else:
                            for kc in range(KO):
                                if half_mm and kc >= KO // 2:
                                    continue
                                for n in range(NT):
                                    nc.tensor.matmul(
                                        pss[n][:],
                                        xT_cur[:, kc],
                                        w_eff[:, kc, n * NF : (n + 1) * NF],
                                        start=(kc == 0),
                                        stop=(kc == (KO // 2 if half_mm else KO) - 1),
                                    )
                            for n in range(NT):
                                nc.any.tensor_copy(
                                    o_sb[:, n * NF : (n + 1) * NF], pss[n][:]
                                )
                            nc.sync.dma_start(
                                out_ap[m * P : (m + 1) * P, :], o_sb[:]
                            )

    nc.compile()
    return nc


def kernel(x: np.ndarray, weight: np.ndarray, disha: np.ndarray) -> np.ndarray:
    assert x.shape == (B, S, D_IN) and weight.shape == (D_OUT, D_IN)
    assert disha.shape == (R, D_OUT)
    x = np.ascontiguousarray(x, dtype=np.float32)
    weight = np.ascontiguousarray(weight, dtype=np.float32)
    disha = np.ascontiguousarray(disha, dtype=np.float32)
    in_maps = [
        {"x": x[c], "w": weight, "disha": disha} for c in range(N_CORES)
    ]
    # The axon-proxied exec occasionally dies with NRT_EXEC_UNIT_UNRECOVERABLE
    # on an otherwise-good NEFF; retry a couple of times with a fresh build.
    last_exc = None
    for attempt in range(3):
        try:
            nc = build_bass()
            res = run_bass_kernel_spmd(
                nc, in_maps, core_ids=list(range(N_CORES))
            )
            break
        except Exception as e:  # noqa: BLE001
            last_exc = e
            import time as _time

            _time.sleep(5.0 * (attempt + 1))
    else:
        raise last_exc
    out = np.stack([res.results[c]["out"] for c in range(N_CORES)], axis=0)
    return out


if __name__ == "__main__":
    rng = np.random.default_rng(0)
    x = rng.standard_normal((B, S, D_IN), dtype=np.float32)
    w = (rng.standard_normal((D_OUT, D_IN), dtype=np.float32) / 32.0).astype(
        np.float32
    )
    d = (rng.standard_normal((R, D_OUT), dtype=np.float32) * 0.01).astype(np.float32)
    out = kernel(x=x, weight=w, disha=d)
    print(out.shape, out.dtype)


# revision 16
# speedup vs baseline: 1.2301x; 1.1486x over previous
"""Trainium2 Bass kernel for nn_BoneLinear: out = x @ W^T + pooled(x) @ disha.

Identity used: pooled(x) @ disha == x @ A where A[j, o] = disha[j % 64, o]
(vertical tiling of disha).  So the whole module is one dense matmul:
    out = x @ (W^T + tile(disha, 16))

Per-core pipeline (all 8 cores run this over their own batch shard):
  1. Setup: load W naturally, PE-transpose it (4 transposes packed per PSUM
     bank as one accumulation group), add the partition-tiled disha, and round
     to fp16 -> W_eff^T resident in SBUF [128, 8, 1024].
  2. Steady state, software-pipelined over 32 token tiles: HWDGE-load x in
     1 MB pairs -> cast f32->fp16 -> PE-transpose each [128,128] chunk
     (packed 4/bank) -> DVE copy to SBUF -> 16 accumulating fp16 matmuls
     (N=512) -> ACT/DVE copies PSUM->SBUF -> HWDGE store.  Loads run ~4
     tiles ahead and transposes 2 ahead so the PE stream (the bottleneck:
     512*213ns matmuls + 256*53ns transposes per pass) never waits on DMA;
     the final tile runs its matmuls n-outer and stores in two half-tiles
     so the post-loop drain is ~2 us instead of ~5.
  3. A post-legalize pass (_dedup_ldweights) deletes the 248 redundant
     InstLdweights that reload an identical stationary operand for the
     second matmul of each (tile, kc) pair: the PE array already holds the
     weights, and each such load costs ~45ns of serial PE time on hardware
     (measured 171us -> 162.6us from this alone; correctness bit-identical).
  fp16 operands measured relmax ~3.3e-4 vs the fp32 reference.

Sharding: pure data-parallel over batch (B=8 -> one batch element per core).
Each core reads its x shard [4096, 1024], full weight and disha, and writes
its output shard [4096, 1024].  No collectives.
"""

import sys
import os

for _p in ("/opt/trn_rl_repo", "/root/.axon_site/_ro/trn_rl_repo"):
    if os.path.isdir(_p) and _p not in sys.path:
        sys.path.insert(0, _p)

import numpy as np

import concourse.bass as bass
import concourse.mybir as mybir
import concourse.tile as tile
from concourse import bacc
from concourse.bass_utils import run_bass_kernel_spmd
from concourse.masks import make_identity


def _dedup_ldweights(ordered):
    """Drop InstLdweights that reload the stationary operand already in the
    PE array (adjacent matmuls sharing one lhsT: bass emits a fresh
    Ldweights per matmul, but the array still holds the weights, and the
    pull-ahead reorder window never runs ahead of an in-flight full-array
    matmul, so the second load is pure serial PE time ~53ns).  Only drops a
    load when the PE stream between the two loads is matmuls only; dep edges
    of the dropped load are merged into the following matmul."""
    for bb, insts in ordered.items():
        cur_sig = None
        drop = {}
        pe_ixs = [
            ix for ix, i in enumerate(insts) if i.engine == mybir.EngineType.PE
        ]
        for k, ix in enumerate(pe_ixs):
            i = insts[ix]
            t = type(i).__name__
            if t == "InstLdweights":
                sig = (
                    repr(i.ins[0]),
                    str(i.is_transpose), str(i.perf_mode),
                    str(i.tile_position), str(i.tile_size),
                )
                if sig == cur_sig and k + 1 < len(pe_ixs):
                    nxt = insts[pe_ixs[k + 1]]
                    if type(nxt).__name__ == "InstMatmult":
                        drop[ix] = nxt
                else:
                    cur_sig = sig
            elif t == "InstMatmult":
                pass  # does not disturb loaded weights
            else:
                cur_sig = None
        if drop:
            for ix, nxt in drop.items():
                nxt.add_sync_dependencies_from(
                    insts[ix].sync_dependency_set_copy()
                )
                nxt.add_nosync_dependencies_from(
                    insts[ix].nosync_dependency_set_copy()
                )
            ordered[bb] = [i for ix, i in enumerate(insts) if ix not in drop]
    return ordered

# Problem shapes (hardcoded per contract)
B, S, D_IN, D_OUT, R = 8, 4096, 1024, 1024, 64
N_CORES = 8
P = 128
KO = D_IN // P          # 8 contraction chunks of 128
OC = D_OUT // P         # 8 output chunks of 128 (for W transpose)
MT = S // P             # 32 token tiles per core
NF = 512                # matmul moving free dim (one PSUM bank of fp32)
NT = D_OUT // NF        # 2 n-tiles

F32 = mybir.dt.float32
F16 = mybir.dt.float16
MM_DT = F16


def build_bass(reps: int = 1, loop: int = 1, n_outer: bool = False,
               eag: int = 4, ta: int = 2, half_mm: bool = False,
               dedup_ldw: bool = True, staggered: bool = True):
    """reps>1 (python-unrolled) or loop>1 (hardware For_i) repeat the
    steady-state compute inside the NEFF; used only for wall-clock
    differencing in benchmarks (the graded kernel uses reps=1, loop=1).
    n_outer / half_mm are timing-diagnostic knobs only."""
    _orig_legalize = tile.tile_legalize
    if dedup_ldw:
        def _legalize_and_dedup(ordered, nc_):
            return _dedup_ldweights(_orig_legalize(ordered, nc_))

        tile.tile_legalize = _legalize_and_dedup
    try:
        nc = _build_bass_inner(reps, loop, n_outer, eag, ta, half_mm, staggered)
    finally:
        tile.tile_legalize = _orig_legalize
    return nc


def _build_bass_inner(reps, loop, n_outer, eag, ta, half_mm, staggered=False):
    nc = bacc.Bacc("TRN2", target_bir_lowering=False, debug=False, num_devices=1)
    x_ap = nc.dram_tensor("x", [S, D_IN], F32, kind="ExternalInput").ap()
    w_ap = nc.dram_tensor("w", [D_OUT, D_IN], F32, kind="ExternalInput").ap()
    d_ap = nc.dram_tensor("disha", [R, D_OUT], F32, kind="ExternalInput").ap()
    out_ap = nc.dram_tensor("out", [S, D_OUT], F32, kind="ExternalOutput").ap()

    GRP = NF // P  # 4 transposes packed per PSUM bank

    with tile.TileContext(nc) as tc:
        with (
            tc.tile_pool(name="const", bufs=1) as const,
            tc.tile_pool(name="wp", bufs=1) as wpool,
            tc.tile_pool(name="xp", bufs=max(6, eag + 2)) as xpool,
            tc.tile_pool(name="xh", bufs=max(5, eag + 1)) as xhpool,
            tc.tile_pool(name="xtp", bufs=max(5, ta + 3)) as xtpool,
            tc.tile_pool(name="op", bufs=4) as opool,
            tc.tile_pool(name="pstp", bufs=4, space="PSUM") as psum_tp,
            tc.tile_pool(name="psacc", bufs=2, space="PSUM") as psum_acc,
        ):
            ident = const.tile([P, P], MM_DT)
            make_identity(nc, ident)

            # disha tiled twice on partitions: disha2[p, :] = disha[p % 64, :]
            disha2f = const.tile([P, D_OUT], F32)
            nc.sync.dma_start(disha2f[0:R, :], d_ap[:, :])
            nc.sync.dma_start(disha2f[R : 2 * R, :], d_ap[:, :])
            cast_copy = nc.any.tensor_copy
            disha2 = const.tile([P, D_OUT], MM_DT)
            cast_copy(disha2[:], disha2f[:])

            # Build W_eff^T[p + 128*kc, oc*128 + q] = W[q(within oc), p(of kc)] + disha2[p]
            w_eff = wpool.tile([P, KO, D_OUT], MM_DT)
            with tc.tile_pool(name="wnat", bufs=1) as wnat_pool:
                w_nat = wnat_pool.tile([P, OC, D_IN], F32)
                w_nath = wnat_pool.tile([P, OC, D_IN], MM_DT)
                w_src = w_ap.rearrange("(oc p) d -> p oc d", p=P)
                for kc in range(KO):
                    nc.sync.dma_start(
                        w_nat[:, :, kc * P : (kc + 1) * P],
                        w_src[:, :, kc * P : (kc + 1) * P],
                    )
                    cast_copy(
                        w_nath[:, :, kc * P : (kc + 1) * P],
                        w_nat[:, :, kc * P : (kc + 1) * P],
                    )
                for kc in range(KO):
                    for og in range(OC // GRP):
                        pst = psum_tp.tile([P, NF], MM_DT, tag="tp")
                        for j in range(GRP):
                            oc = og * GRP + j
                            nc.tensor.matmul(
                                pst[:, j * P : (j + 1) * P],
                                w_nath[:, oc, kc * P : (kc + 1) * P],
                                ident[:],
                                is_transpose=True,
                                start=(j == 0),
                                stop=(j == GRP - 1),
                            )
                        nc.vector.tensor_add(
                            w_eff[:, kc, og * NF : (og + 1) * NF],
                            pst[:],
                            disha2[:, og * NF : (og + 1) * NF],
                        )

            # Main loop over token tiles
            import contextlib

            loop_cm = (
                tc.For_i(0, loop, 1, staggered_reset=staggered)
                if loop > 1
                else contextlib.nullcontext()
            )
            with loop_cm:
                for rep in range(reps):

                    PW = 2
                    NP = MT // PW

                    def emit_load_pair(p, rep=rep):
                        """DMA a 1MB pair of token tiles and cast to fp16."""
                        x_t = xpool.tile(
                            [P, PW, D_IN], F32, tag="x_t", name=f"x_{rep}_{p}"
                        )
                        nc.sync.dma_start(
                            x_t[:],
                            x_ap[p * PW * P : (p + 1) * PW * P, :].rearrange(
                                "(two p) d -> p two d", two=PW
                            ),
                        )
                        x_h = xhpool.tile(
                            [P, PW, D_IN], MM_DT, tag="x_h", name=f"xh_{rep}_{p}"
                        )
                        cast_copy(x_h[:], x_t[:])
                        return x_h

                    def emit_transpose(x_h, m, rep=rep):
                        """PE-transpose token tile m (half m%PW of its pair)."""
                        t = m % PW
                        xT = xtpool.tile(
                            [P, KO, P], MM_DT, tag="xT", name=f"xT_{rep}_{m}"
                        )
                        for g in range(KO // GRP):
                            pst = psum_tp.tile(
                                [P, NF], MM_DT, tag="tp", name=f"tp_{rep}_{m}_{g}"
                            )
                            for j in range(GRP):
                                kc = g * GRP + j
                                nc.tensor.matmul(
                                    pst[:, j * P : (j + 1) * P],
                                    x_h[:, t, kc * P : (kc + 1) * P],
                                    ident[:],
                                    is_transpose=True,
                                    start=(j == 0),
                                    stop=(j == GRP - 1),
                                )
                            nc.vector.tensor_copy(
                                xT[:, g * GRP : (g + 1) * GRP], pst[:]
                            )
                        return xT

                    eagp = max(1, (eag + PW - 1) // PW)
                    xh_buf = {}
                    xT_buf = {}
                    for j in range(min(eagp, NP)):
                        xh_buf[j] = emit_load_pair(j)
                    for j in range(min(ta, MT)):
                        xT_buf[j] = emit_transpose(xh_buf[j // PW], j)

                    for m in range(MT):
                        if m % PW == 0 and m // PW + eagp < NP:
                            xh_buf[m // PW + eagp] = emit_load_pair(
                                m // PW + eagp
                            )
                        if m + ta < MT:
                            xT_buf[m + ta] = emit_transpose(
                                xh_buf[(m + ta) // PW], m + ta
                            )
                        xT_cur = xT_buf.pop(m)
                        if m % PW == PW - 1:
                            xh_buf.pop(m // PW, None)

                        last = m == MT - 1
                        o_sb = opool.tile(
                            [P, D_OUT], F32, tag="o", name=f"o_{rep}_{m}"
                        )
                        pss = [
                            psum_acc.tile(
                                [P, NF], F32, tag=f"acc{n}", name=f"acc_{rep}_{m}_{n}"
                            )
                            for n in range(NT)
                        ]
                        if last or n_outer:
                            # n-outer: finish bank n=0's kc chain first so its
                            # copy+store overlaps the n=1 matmuls -> short drain
                            for n in range(NT):
                                for kc in range(KO):
                                    if half_mm and kc >= KO // 2:
                                        continue
                                    nc.tensor.matmul(
                                        pss[n][:],
                                        xT_cur[:, kc],
                                        w_eff[:, kc, n * NF : (n + 1) * NF],
                                        start=(kc == 0),
                                        stop=(kc == (KO // 2 if half_mm else KO) - 1),
                                    )
                                nc.any.tensor_copy(
                                    o_sb[:, n * NF : (n + 1) * NF], pss[n][:]
                                )
                                nc.sync.dma_start(
                                    out_ap[m * P : (m + 1) * P, n * NF : (n + 1) * NF],
                                    o_sb[:, n * NF : (n + 1) * NF],
                                )
                        else:
                            for kc in range(KO):
                                if half_mm and kc >= KO // 2:
                                    continue
                                for n in range(NT):
                                    nc.tensor.matmul(
                                        pss[n][:],
                                        xT_cur[:, kc],
                                        w_eff[:, kc, n * NF : (n + 1) * NF],
                                        start=(kc == 0),
                                        stop=(kc == (KO // 2 if half_mm else KO) - 1),
                                    )
                            for n in range(NT):
                                nc.any.tensor_copy(
                                    o_sb[:, n * NF : (n + 1) * NF], pss[n][:]
                                )
                            nc.sync.dma_start(
                                out_ap[m * P : (m + 1) * P, :], o_sb[:]
                            )

    nc.compile()
    return nc


def kernel(x: np.ndarray, weight: np.ndarray, disha: np.ndarray) -> np.ndarray:
    assert x.shape == (B, S, D_IN) and weight.shape == (D_OUT, D_IN)
    assert disha.shape == (R, D_OUT)
    x = np.ascontiguousarray(x, dtype=np.float32)
    weight = np.ascontiguousarray(weight, dtype=np.float32)
    disha = np.ascontiguousarray(disha, dtype=np.float32)
    in_maps = [
        {"x": x[c], "w": weight, "disha": disha} for c in range(N_CORES)
    ]
    # The axon-proxied exec occasionally dies with NRT_EXEC_UNIT_UNRECOVERABLE
    # on an otherwise-good NEFF; retry a couple of times with a fresh build.
    last_exc = None
    for attempt in range(3):
        try:
            nc = build_bass()
            res = run_bass_kernel_spmd(
                nc, in_maps, core_ids=list(range(N_CORES))
            )
            break
        except Exception as e:  # noqa: BLE001
            last_exc = e
            import time as _time

            _time.sleep(5.0 * (attempt + 1))
    else:
        raise last_exc
    out = np.stack([res.results[c]["out"] for c in range(N_CORES)], axis=0)
    return out


if __name__ == "__main__":
    rng = np.random.default_rng(0)
    x = rng.standard_normal((B, S, D_IN), dtype=np.float32)
    w = (rng.standard_normal((D_OUT, D_IN), dtype=np.float32) / 32.0).astype(
        np.float32
    )
    d = (rng.standard_normal((R, D_OUT), dtype=np.float32) * 0.01).astype(np.float32)
    out = kernel(x=x, weight=w, disha=d)
    print(out.shape, out.dtype)
